# revision 14
# baseline (speedup 1.0000x reference)
"""GQA causal attention (B=2, S=2048, H=2048, 32 Q heads / 8 KV heads, hd=64)
as an 8-way tensor-parallel Trainium2 Bass kernel.

Sharding: heads. Each NeuronCore gets 4 Q heads + their KV head (Wq/Wk/Wv
column slices, Wo row slice), computes a partial output over the full batch,
and the host sums the 8 bf16 partials (the Wo all-reduce done host-side).

v2 design (vs the fp32r baseline): everything bf16 on the PE, and the whole
kernel is ONE software-pipelined loop over 8 superblocks of 512 query
positions.  In slot i the instruction stream interleaves four stages --
ht prefetch for block i+2, projections of block i+1, attention of block i,
output projection of block i-1 -- so the tensor engine always has an
independent matmul ready and stays at its full (ramped) clock.  Causal
structure is exploited at matmul granularity: scores/exp/AV only touch
columns q >= k.

Per-core dataflow (d-major, no activation transposes except V):
    Q_T  = (Wq_c * scale)^T @ hidden^T          [256, B*S]   (heads stacked)
    K_T  = Wk_c^T @ hidden^T  (rows 0-63, duplicated to 64-127 for odd heads)
    V    = PE-transpose(Wv_c^T @ hidden^T)      [keys, 64] stored [V|1|V]
    S_T[k,q] = K_T(chunk)^T x Q_T               causal chunks only
    P_T  = exp(S_T + tri on diagonal chunks)    bf16
    ctx_aug = [V|1]^T @ P_T                     even heads -> psum rows 0-64
              [1|V]^T @ P_T                     odd heads  -> psum rows 63-127
    ctx  = ctx_aug * bcast(1/denom)             denom recip via [4,128] DVE
    out_partial = ctx^T @ Wo_c                  [B*S, 2048] bf16
"""

import sys

for _p in ("/root/.axon_site", "/root/.axon_site/_ro/trn_rl_repo",
           "/root/.axon_site/_ro/pypackages", "/opt/trn_rl_repo", "/opt/pypackages"):
    if _p not in sys.path:
        sys.path.append(_p)

from contextlib import ExitStack

import numpy as np

import concourse.bass as bass  # noqa: F401
import concourse.tile as tile
from concourse import bacc, mybir
from concourse.bass_utils import run_bass_kernel_spmd

F32 = mybir.dt.float32
BF16 = mybir.dt.bfloat16
P = 128
KC = 128
QT = 512
N_CORES = 8
HD = 64
NEG = -1e9

TRACE = False            # test harness flips this for NTFF profiling
TRACE_CORES = None
LAST_RESULT = None       # BassKernelResults of the last run (for the harness)

_nc_cache = {}


def build_attn_core(B=2, S=2048, H=2048, NHL=4, mask_mode="causal", debug_dump=False):
    """Build + bass-compile the per-core program.

    DRAM inputs (per core):
      ht  [H, B*S] bf16   hidden transposed      wq [H, NHL*HD] bf16 (pre-scaled)
      wkv [H, 2*HD] bf16  [Wk_c | Wv_c]          wo [NHL*HD, H] bf16
      tri [KC, KC] f32    transposed causal block mask (tri[k,q]=0 iff k<=q)
      maskt [B, S, S] f32 (only mask_mode=="full") additive mask transposed
    Output: out_p [B*S, H] bf16.
    """
    NQ = B * S
    CL = NHL * HD                       # 256 q-head cols per core
    assert H % P == 0 and S % QT == 0
    NHC = H // P                        # 16 contraction chunks
    NCC = CL // P                       # 2 head-pair groups
    QPB = S // QT                       # 4 q-blocks per batch
    NBLK = B * QPB                      # 8 superblocks
    KPB = S // KC                       # 16 key chunks per batch
    DPT = QT // KC                      # 4 key chunks per q-block
    EXP = mybir.ActivationFunctionType.Exp
    CPY = mybir.ActivationFunctionType.Copy

    nc = bacc.Bacc("TRN2", target_bir_lowering=False, debug=False)

    ht = nc.dram_tensor("ht", [H, NQ], BF16, kind="ExternalInput").ap()
    wq = nc.dram_tensor("wq", [H, CL], BF16, kind="ExternalInput").ap()
    wkv = nc.dram_tensor("wkv", [H, 2 * HD], BF16, kind="ExternalInput").ap()
    wo = nc.dram_tensor("wo", [CL, H], BF16, kind="ExternalInput").ap()
    tri = nc.dram_tensor("tri", [KC, KC], F32, kind="ExternalInput").ap()
    if mask_mode == "full":
        maskt = nc.dram_tensor("maskt", [B, S, S], F32, kind="ExternalInput").ap()
    out_p = nc.dram_tensor("out_p", [NQ, H], BF16, kind="ExternalOutput").ap()
    dscr = nc.dram_tensor("dscr", [NBLK * NHL, QT], F32, kind="Internal").ap()
    if debug_dump:
        dbg_qt = nc.dram_tensor("dbg_qt", [NCC, P, NQ], BF16, kind="ExternalOutput").ap()
        dbg_kt = nc.dram_tensor("dbg_kt", [P, NQ], BF16, kind="ExternalOutput").ap()
        dbg_v = nc.dram_tensor("dbg_v", [P, NQ // KC, HD + 1], BF16, kind="ExternalOutput").ap()
        dbg_ctx = nc.dram_tensor("dbg_ctx", [NBLK, P, NCC, QT], BF16, kind="ExternalOutput").ap()
        dbg_rb = nc.dram_tensor("dbg_rb", [NBLK * NHL, QT], F32, kind="ExternalOutput").ap()
    dscr2 = nc.dram_tensor("dscr2", [NBLK * NHL, QT], F32, kind="Internal").ap()

    ht_r = ht.rearrange("(o p) m -> p o m", p=P)      # [128, 16, 4096]

    with tile.TileContext(nc) as tc, ExitStack() as ctx:
        # ---- persistent SBUF ----
        pers = ctx.enter_context(tc.tile_pool(name="pers", bufs=1))
        wq_sb = pers.tile([P, NHC, CL], BF16, tag="wq")
        wkv_sb = pers.tile([P, NHC, 2 * HD], BF16, tag="wkv")
        wo_sb = pers.tile([P, NCC, H], BF16, tag="wo")
        tri_sb = pers.tile([KC, KC], F32, tag="tri")
        # weight loads split so the first projection can start early
        wq_r = wq.rearrange("(o p) m -> p o m", p=P)
        wkv_r = wkv.rearrange("(o p) m -> p o m", p=P)
        wo_r = wo.rearrange("(o p) m -> p o m", p=P)
        for g in range(8):
            nc.sync.dma_start(wq_sb[:, g * 2:(g + 1) * 2, :],
                              wq_r[:, g * 2:(g + 1) * 2, :])
        for g in range(2):
            nc.sync.dma_start(wkv_sb[:, g * 8:(g + 1) * 8, :],
                              wkv_r[:, g * 8:(g + 1) * 8, :])
        for g in range(2):
            for g2 in range(2):
                nc.sync.dma_start(wo_sb[:, g, g2 * (H // 2):(g2 + 1) * (H // 2)],
                                  wo_r[:, g, g2 * (H // 2):(g2 + 1) * (H // 2)])
        nc.sync.dma_start(tri_sb[:], tri)

        # identity (bf16) for PE transposes of V
        ident = pers.tile([P, P], BF16, tag="ident")
        nc.gpsimd.memset(ident[:], 1.0)
        nc.gpsimd.affine_select(
            out=ident[:], in_=ident[:],
            compare_op=mybir.AluOpType.is_equal, fill=0.0,
            base=0, pattern=[[-1, P]], channel_multiplier=1,
        )

        qt_sb = [pers.tile([P, NQ], BF16, tag=f"qt{c}", name=f"qt{c}")
                 for c in range(NCC)]
        kt_sb = pers.tile([P, NQ], BF16, tag="kt")          # [K_T ; K_T]
        v_sb = pers.tile([P, NQ // KC, HD + 1], BF16, tag="v")      # [V|1]
        nc.gpsimd.memset(v_sb[:, :, HD], 1.0)
        ctx_sb = pers.tile([P, 2, NCC, QT], BF16, tag="ctx")

        # ---- SBUF pools ----
        hpool = ctx.enter_context(tc.tile_pool(name="hpool", bufs=2))
        vt_pool = ctx.enter_context(tc.tile_pool(name="vtp", bufs=2))
        pt_pool = ctx.enter_context(tc.tile_pool(name="ptp", bufs=4))
        dpool = ctx.enter_context(tc.tile_pool(name="dpool", bufs=3))
        bcpool = ctx.enter_context(tc.tile_pool(name="bcp", bufs=2))
        obpool = ctx.enter_context(tc.tile_pool(name="obp", bufs=3))
        if mask_mode == "full":
            mpool = ctx.enter_context(tc.tile_pool(name="mpool", bufs=4))

        # ---- PSUM pool (tags: pq 1, pkv 1, sps 2, cps 2, pow 2) ----
        psum = ctx.enter_context(tc.tile_pool(name="psum", bufs=1, space="PSUM"))

        ht_tiles = {}                   # blk -> [4 x tile [128,4,512]]

        # ================= stream generators =================
        # Each stream yields (kind, closure); emission interleaves streams.

        def prefetch_stream(blk):
            """Issue the 4 coarse ht DMAs for superblock blk."""
            g0 = blk * QT
            tiles = []
            for g in range(4):
                h4 = hpool.tile([P, 4, QT], BF16, tag=f"h{g}", name=f"h4_{g}")
                tiles.append(h4)
            ht_tiles[blk] = tiles

            def mk(g):
                def emit():
                    nc.sync.dma_start(tiles[g][:],
                                      ht_r[:, g * 4:(g + 1) * 4, g0:g0 + QT])
                return emit
            for g in range(4):
                yield ('dma', mk(g))

        def proj_stream(blk):
            """Projections of superblock blk -> qt_sb/kt_sb/v_sb columns."""
            g0 = blk * QT
            hts = ht_tiles[blk]
            pq0 = psum.tile([P, QT], F32, tag="pq0", bufs=1, name="pq0")
            pq1 = psum.tile([P, QT], F32, tag="pq1", bufs=1, name="pq1")
            pkv = psum.tile([P, QT], F32, tag="pkv", bufs=1, name="pkv")

            def mk_mm(hc, pq0=pq0, pq1=pq1, pkv=pkv):
                def emit():
                    mv = hts[hc // 4][:, hc % 4, :]
                    fl = dict(start=(hc == 0), stop=(hc == NHC - 1))
                    nc.tensor.matmul(pq0[:], wq_sb[:, hc, 0:P], mv, **fl)
                    nc.tensor.matmul(pq1[:], wq_sb[:, hc, P:CL], mv, **fl)
                    nc.tensor.matmul(pkv[:], wkv_sb[:, hc, :], mv, **fl)
                return emit
            for hc in range(NHC):
                yield ('mm3', mk_mm(hc))

            vtmp = vt_pool.tile([P, QT], BF16, tag="vt", name="vtmp")

            def drain(pq0=pq0, pq1=pq1, pkv=pkv, vtmp=vtmp):
                nc.vector.tensor_copy(qt_sb[0][:, g0:g0 + QT], pq0[:])
                nc.vector.tensor_copy(qt_sb[1][:, g0:g0 + QT], pq1[:])
                nc.vector.tensor_copy(kt_sb[:HD, g0:g0 + QT], pkv[:HD, :])
                nc.vector.tensor_copy(vtmp[HD:2 * HD, :], pkv[HD:2 * HD, :])
            yield ('drain', drain)

            def mk_tr(j, vtmp=vtmp):
                kcg = g0 // KC + j

                def emit():
                    tp = psum.tile([P, HD], BF16, tag="sps", bufs=2, name="tp")
                    nc.tensor.transpose(
                        tp[:, :HD],
                        vtmp[HD:2 * HD, j * KC:(j + 1) * KC],
                        ident[HD:2 * HD, HD:2 * HD],
                    )
                    nc.scalar.activation(v_sb[:, kcg, :HD], tp[:, :HD], CPY)
                return emit
            for j in range(DPT):
                yield ('tr', mk_tr(j))
            # duplicate K rows 0-63 -> 64-127 for odd heads
            yield ('dma', lambda: nc.gpsimd.dma_start(
                kt_sb[HD:2 * HD, g0:g0 + QT], kt_sb[:HD, g0:g0 + QT]))

        def attn_stream(blk):
            """Attention of superblock blk into ctx_sb[:, blk%2]."""
            b, qtb = blk // QPB, blk % QPB
            g0 = blk * QT
            ib = blk % 2
            nkc = (qtb + 1) * DPT if mask_mode == "causal" else KPB
            for h in range(NHL):
                hb = (h % 2) * HD
                cc = h // 2
                even = (h % 2 == 0)
                cps = psum.tile([P, QT], F32, tag="cps", bufs=2, name="cps")
                pend = []   # deferred AV emissions (one-unit lag)

                def emit_av(item, cps=cps, nkc=nkc):
                    akc, alo, apt, akcg = item
                    nc.tensor.matmul(cps[0:HD + 1, alo:], v_sb[:, akcg, :],
                                     apt[:, alo:],
                                     start=(akc == 0), stop=(akc == nkc - 1))

                def mk_unit(kc, hb=hb, cc=cc, pend=pend, emit_av=emit_av, b=b):
                    kcg = b * KPB + kc
                    do = kc * KC - qtb * QT if mask_mode == "causal" else -1
                    lo = max(do, 0)

                    def emit():
                        sps = psum.tile([P, QT], F32, tag="sps", bufs=2,
                                        name="sps")
                        nc.tensor.matmul(
                            sps[:, lo:],
                            kt_sb[hb:hb + HD, kcg * KC:(kcg + 1) * KC],
                            qt_sb[cc][hb:hb + HD, g0 + lo:g0 + QT],
                            start=True, stop=True,
                        )
                        if pend:
                            emit_av(pend.pop(0))
                        if mask_mode == "full":
                            mt = mpool.tile([KC, QT], F32, tag="mt", name="mt")
                            nc.sync.dma_start(
                                mt[:], maskt[b, kc * KC:(kc + 1) * KC,
                                             (g0 - b * S):(g0 - b * S) + QT])
                            nc.vector.tensor_add(sps[:], sps[:], mt[:])
                        elif do >= 0:
                            nc.vector.tensor_add(
                                sps[:, do:do + KC], sps[:, do:do + KC], tri_sb[:])
                        pt = pt_pool.tile([P, QT], BF16, tag="pt", name="pt")
                        nc.scalar.activation(pt[:, lo:], sps[:, lo:], EXP)
                        pend.append((kc, lo, pt, kcg))
                    return emit

                for kc in range(nkc):
                    yield ('attn', mk_unit(kc))

                def finalize(h=h, cc=cc, even=even, cps=cps, pend=pend,
                             emit_av=emit_av, ib=ib, blk=blk):
                    while pend:
                        emit_av(pend.pop(0))
                    # normalize: denom row -> dram -> [4,128] -> recip -> dram
                    # -> [1,512] -> broadcast.  (SBUF partition-reshape DMAs
                    # are illegal; the DRAM bounce is the legal spelling.)
                    hh = blk * NHL + h
                    den = dpool.tile([P, QT], F32, tag="den", name="den")
                    nc.scalar.activation(den[HD:HD + 1, :],
                                         cps[HD:HD + 1, :], CPY)
                    nc.gpsimd.dma_start(dscr[hh:hh + 1, :], den[HD:HD + 1, :])
                    dh = dpool.tile([4, KC], F32, tag="dh", name="dh")
                    nc.gpsimd.dma_start(
                        dh[:],
                        dscr[hh:hh + 1, :].rearrange("o (a b) -> (o a) b", a=4))
                    rc = dpool.tile([4, KC], F32, tag="rc", name="rc")
                    nc.vector.reciprocal(rc[:], dh[:])
                    nc.gpsimd.dma_start(
                        dscr2[hh:hh + 1, :].rearrange("o (a b) -> (o a) b", a=4),
                        rc[:])
                    rb = dpool.tile([1, QT], F32, tag="rb", name="rb")
                    nc.gpsimd.dma_start(rb[:], dscr2[hh:hh + 1, :])
                    if debug_dump:
                        nc.sync.dma_start(dbg_rb[hh:hh + 1, :], rb[:])
                    bc = bcpool.tile([P, QT], F32, tag="bc", name="bc")
                    nc.gpsimd.partition_broadcast(bc[0:HD, :], rb[:])
                    if even:
                        nc.vector.tensor_mul(ctx_sb[0:HD, ib, cc, :],
                                             cps[0:HD, :], bc[0:HD, :])
                    else:
                        ctmp = bcpool.tile([HD, QT], BF16, tag="ctmp",
                                           name="ctmp")
                        nc.vector.tensor_mul(ctmp[:], cps[0:HD, :], bc[0:HD, :])
                        nc.gpsimd.dma_start(ctx_sb[HD:2 * HD, ib, cc, :],
                                            ctmp[:])
                yield ('fin', finalize)

        def wo_stream(blk):
            """Output projection of superblock blk from ctx_sb[:, blk%2]."""
            ib = blk % 2
            r0 = blk * QT
            ET = 512
            last_blk = (blk == NBLK - 1)
            if debug_dump:
                yield ('dbg', lambda: nc.sync.dma_start(
                    dbg_ctx[blk], ctx_sb[:, ib, :, :]))
            for qc in range(QT // P):
                ob = obpool.tile([P, H], BF16, tag="ob", name="ob")

                def mk_unit(et, qc=qc, ob=ob):
                    def emit():
                        if last_blk:
                            # attention is done: cps/sps banks are free, so
                            # rotate through 3 banks to avoid drain stalls
                            tg = ["pow", "cps", "sps"][(qc * 4 + et) % 3]
                            bufs = {"pow": 1, "cps": 2, "sps": 2}[tg]
                        else:
                            tg, bufs = "pow", 1
                        po = psum.tile([P, ET], F32, tag=tg, bufs=bufs,
                                       name="po")
                        for cc2 in range(NCC):
                            nc.tensor.matmul(
                                po[:],
                                ctx_sb[:, ib, cc2, qc * P:(qc + 1) * P],
                                wo_sb[:, cc2, et * ET:(et + 1) * ET],
                                start=(cc2 == 0), stop=(cc2 == NCC - 1),
                            )
                        dst = ob[:, et * ET:(et + 1) * ET]
                        nc.vector.tensor_copy(dst, po[:])
                        if last_blk:
                            nc.gpsimd.dma_start(
                                out_p[r0 + qc * P:r0 + (qc + 1) * P,
                                      et * ET:(et + 1) * ET], dst)
                        elif et % 2 == 1:
                            nc.gpsimd.dma_start(
                                out_p[r0 + qc * P:r0 + (qc + 1) * P,
                                      (et - 1) * ET:(et + 1) * ET],
                                ob[:, (et - 1) * ET:(et + 1) * ET])
                    return emit
                for et in range(H // ET):
                    yield ('wo', mk_unit(et))

        # ================= merge + emit =================
        def merge(streams):
            """Proportional interleave of unit streams (virtual-time merge)."""
            lists = [list(s) for s in streams if s is not None]
            lists = [l for l in lists if l]
            idx = [0] * len(lists)
            while True:
                best, bestv = -1, 2.0
                for j, l in enumerate(lists):
                    if idx[j] < len(l):
                        v = (idx[j] + 0.5) / len(l)
                        if v < bestv:
                            best, bestv = j, v
                if best < 0:
                    break
                lists[best][idx[best]][1]()
                idx[best] += 1

        # prologue: fine-grained ht loads for blocks 0-1 so the first
        # matmuls start early, then projections of blocks 0 and 1.
        for blk0 in range(2):
            tiles0 = [hpool.tile([P, 4, QT], BF16, tag=f"h{g}", name=f"h4p_{g}")
                      for g in range(4)]
            ht_tiles[blk0] = tiles0
            for g in range(4):
                for j in range(4):
                    nc.sync.dma_start(tiles0[g][:, j, :],
                                      ht_r[:, g * 4 + j, blk0 * QT:(blk0 + 1) * QT])
        for _, emit in prefetch_stream(2):
            emit()
        for _, emit in proj_stream(0):
            emit()
        for _, emit in proj_stream(1):
            emit()
        # main loop: attention trails projections by two slots
        for i in range(NBLK):
            streams = [attn_stream(i)]
            if i + 2 < NBLK:
                streams.append(proj_stream(i + 2))
            if i + 3 < NBLK:
                streams.append(prefetch_stream(i + 3))
            if i - 1 >= 0:
                streams.append(wo_stream(i - 1))
            merge(streams)
        # epilogue
        for _, emit in wo_stream(NBLK - 1):
            emit()
        if debug_dump:
            for c in range(NCC):
                nc.sync.dma_start(dbg_qt[c], qt_sb[c][:])
            nc.sync.dma_start(dbg_kt[:], kt_sb[:])
            nc.sync.dma_start(dbg_v[:], v_sb[:])


    nc.compile()
    return nc


def _detect_mask_mode(m, S):
    if not np.any(m):
        return "zeros"
    b0 = np.asarray(m[0, 0])
    qi = np.arange(S)
    tl = qi[None, :] <= qi[:, None]
    if (b0[tl] == 0.0).all() and (b0[~tl] <= -1e8).all() and (m == b0).all():
        return "causal"
    return "full"


def shard_inputs(hidden_states, attention_mask, Wq, Wk, Wv, Wo, mask_mode):
    import ml_dtypes
    bf16 = ml_dtypes.bfloat16
    B, S, H = hidden_states.shape
    NH = Wq.shape[1] // HD
    NKV = Wk.shape[1] // HD
    NHL = NH // N_CORES
    scale = np.float32(1.0 / np.sqrt(HD))

    ht = np.ascontiguousarray(
        hidden_states.reshape(B * S, H).T).astype(bf16)
    ki = np.arange(KC)
    tri = np.where(ki[:, None] <= ki[None, :], 0.0, NEG).astype(np.float32)
    if mask_mode == "full":
        maskt = np.ascontiguousarray(
            np.asarray(attention_mask)[:, 0].transpose(0, 2, 1).astype(np.float32))

    in_maps = []
    for c in range(N_CORES):
        wq_c = np.ascontiguousarray(
            Wq[:, c * NHL * HD:(c + 1) * NHL * HD] * scale).astype(bf16)
        kv0 = c * (NKV // N_CORES) * HD
        wkv_c = np.ascontiguousarray(np.concatenate(
            [Wk[:, kv0:kv0 + HD], Wv[:, kv0:kv0 + HD]], axis=1)).astype(bf16)
        wo_c = np.ascontiguousarray(
            Wo[c * NHL * HD:(c + 1) * NHL * HD, :]).astype(bf16)
        im = {"ht": ht, "wq": wq_c, "wkv": wkv_c, "wo": wo_c, "tri": tri}
        if mask_mode == "full":
            im["maskt"] = maskt
        in_maps.append(im)
    return in_maps, NHL


def kernel(hidden_states, attention_mask, Wq, Wk, Wv, Wo):
    global LAST_RESULT
    hidden_states = np.asarray(hidden_states, dtype=np.float32)
    attention_mask = np.asarray(attention_mask, dtype=np.float32)
    Wq, Wk, Wv, Wo = (np.asarray(w, dtype=np.float32) for w in (Wq, Wk, Wv, Wo))
    B, S, H = hidden_states.shape

    mask_mode = _detect_mask_mode(attention_mask, S)
    in_maps, NHL = shard_inputs(hidden_states, attention_mask, Wq, Wk, Wv, Wo,
                                mask_mode)

    key = (B, S, H, NHL, mask_mode)
    if key not in _nc_cache:
        _nc_cache[key] = build_attn_core(B=B, S=S, H=H, NHL=NHL,
                                         mask_mode=mask_mode)
    nc = _nc_cache[key]

    res = run_bass_kernel_spmd(nc, in_maps, core_ids=list(range(N_CORES)),
                               trace=TRACE, trace_cores=TRACE_CORES)
    LAST_RESULT = res

    out = res.results[0]["out_p"].astype(np.float32)
    for c in range(1, N_CORES):
        out = out + res.results[c]["out_p"].astype(np.float32)
    return out.reshape(B, S, H)


# revision 15
# speedup vs baseline: 1.0305x; 1.0305x over previous
"""GQA causal attention (B=2, S=2048, H=2048, 32 Q heads / 8 KV heads, hd=64)
as an 8-way tensor-parallel Trainium2 Bass kernel.

Sharding: heads. Each NeuronCore gets 4 Q heads + their KV head (Wq/Wk/Wv
column slices, Wo row slice), computes a partial output over the full batch,
and the host sums the 8 bf16 partials (the Wo all-reduce done host-side).

v2 design (vs the fp32r baseline): everything bf16 on the PE, and the whole
kernel is ONE software-pipelined loop over 8 superblocks of 512 query
positions.  In slot i the instruction stream interleaves four stages --
ht prefetch for block i+2, projections of block i+1, attention of block i,
output projection of block i-1 -- so the tensor engine always has an
independent matmul ready and stays at its full (ramped) clock.  Causal
structure is exploited at matmul granularity: scores/exp/AV only touch
columns q >= k.

Per-core dataflow (d-major, no activation transposes except V):
    Q_T  = (Wq_c * scale)^T @ hidden^T          [256, B*S]   (heads stacked)
    K_T  = Wk_c^T @ hidden^T  (rows 0-63, duplicated to 64-127 for odd heads)
    V    = PE-transpose(Wv_c^T @ hidden^T)      [keys, 64] stored [V|1|V]
    S_T[k,q] = K_T(chunk)^T x Q_T               causal chunks only
    P_T  = exp(S_T + tri on diagonal chunks)    bf16
    ctx_aug = [V|1]^T @ P_T                     even heads -> psum rows 0-64
              [1|V]^T @ P_T                     odd heads  -> psum rows 63-127
    ctx  = ctx_aug * bcast(1/denom)             denom recip via [4,128] DVE
    out_partial = ctx^T @ Wo_c                  [B*S, 2048] bf16
"""

import sys

for _p in ("/root/.axon_site", "/root/.axon_site/_ro/trn_rl_repo",
           "/root/.axon_site/_ro/pypackages", "/opt/trn_rl_repo", "/opt/pypackages"):
    if _p not in sys.path:
        sys.path.append(_p)

from contextlib import ExitStack

import numpy as np

import concourse.bass as bass  # noqa: F401
import concourse.tile as tile
from concourse import bacc, mybir
from concourse.bass_utils import run_bass_kernel_spmd

F32 = mybir.dt.float32
BF16 = mybir.dt.bfloat16
P = 128
KC = 128
QT = 512
N_CORES = 8
HD = 64
NEG = -1e9

TRACE = False            # test harness flips this for NTFF profiling
TRACE_CORES = None
LAST_RESULT = None       # BassKernelResults of the last run (for the harness)

_nc_cache = {}


def build_attn_core(B=2, S=2048, H=2048, NHL=4, mask_mode="causal", debug_dump=False):
    """Build + bass-compile the per-core program.

    DRAM inputs (per core):
      ht  [H, B*S] bf16   hidden transposed      wq [H, NHL*HD] bf16 (pre-scaled)
      wkv [H, 2*HD] bf16  [Wk_c | Wv_c]          wo [NHL*HD, H] bf16
      tri [KC, KC] f32    transposed causal block mask (tri[k,q]=0 iff k<=q)
      maskt [B, S, S] f32 (only mask_mode=="full") additive mask transposed
    Output: out_p [B*S, H] bf16.
    """
    NQ = B * S
    CL = NHL * HD                       # 256 q-head cols per core
    assert H % P == 0 and S % QT == 0
    NHC = H // P                        # 16 contraction chunks
    NCC = CL // P                       # 2 head-pair groups
    QPB = S // QT                       # 4 q-blocks per batch
    NBLK = B * QPB                      # 8 superblocks
    KPB = S // KC                       # 16 key chunks per batch
    DPT = QT // KC                      # 4 key chunks per q-block
    EXP = mybir.ActivationFunctionType.Exp
    CPY = mybir.ActivationFunctionType.Copy

    nc = bacc.Bacc("TRN2", target_bir_lowering=False, debug=False)

    ht = nc.dram_tensor("ht", [H, NQ], BF16, kind="ExternalInput").ap()
    wq = nc.dram_tensor("wq", [H, CL], BF16, kind="ExternalInput").ap()
    wkv = nc.dram_tensor("wkv", [H, 2 * HD], BF16, kind="ExternalInput").ap()
    wo = nc.dram_tensor("wo", [CL, H], BF16, kind="ExternalInput").ap()
    tri = nc.dram_tensor("tri", [KC, KC], F32, kind="ExternalInput").ap()
    if mask_mode == "full":
        maskt = nc.dram_tensor("maskt", [B, S, S], F32, kind="ExternalInput").ap()
    out_p = nc.dram_tensor("out_p", [NQ, H], BF16, kind="ExternalOutput").ap()
    dscr = nc.dram_tensor("dscr", [NBLK * NHL, QT], F32, kind="Internal").ap()
    if debug_dump:
        dbg_qt = nc.dram_tensor("dbg_qt", [NCC, P, NQ], BF16, kind="ExternalOutput").ap()
        dbg_kt = nc.dram_tensor("dbg_kt", [P, NQ], BF16, kind="ExternalOutput").ap()
        dbg_v = nc.dram_tensor("dbg_v", [P, NQ // KC, HD + 1], BF16, kind="ExternalOutput").ap()
        dbg_ctx = nc.dram_tensor("dbg_ctx", [NBLK, P, NCC, QT], BF16, kind="ExternalOutput").ap()
        dbg_rb = nc.dram_tensor("dbg_rb", [NBLK * NHL, QT], F32, kind="ExternalOutput").ap()
    dscr2 = nc.dram_tensor("dscr2", [NBLK * NHL, QT], F32, kind="Internal").ap()

    ht_r = ht.rearrange("(o p) m -> p o m", p=P)      # [128, 16, 4096]

    with tile.TileContext(nc) as tc, ExitStack() as ctx:
        # ---- persistent SBUF ----
        pers = ctx.enter_context(tc.tile_pool(name="pers", bufs=1))
        wq_sb = pers.tile([P, NHC, CL], BF16, tag="wq")
        wkv_sb = pers.tile([P, NHC, 2 * HD], BF16, tag="wkv")
        wo_sb = pers.tile([P, NCC, H], BF16, tag="wo")
        tri_sb = pers.tile([KC, KC], F32, tag="tri")
        # weight loads split so the first projection can start early
        wq_r = wq.rearrange("(o p) m -> p o m", p=P)
        wkv_r = wkv.rearrange("(o p) m -> p o m", p=P)
        wo_r = wo.rearrange("(o p) m -> p o m", p=P)
        for g in range(8):
            nc.sync.dma_start(wq_sb[:, g * 2:(g + 1) * 2, :],
                              wq_r[:, g * 2:(g + 1) * 2, :])
        for g in range(2):
            nc.sync.dma_start(wkv_sb[:, g * 8:(g + 1) * 8, :],
                              wkv_r[:, g * 8:(g + 1) * 8, :])
        for g in range(2):
            for g2 in range(2):
                nc.sync.dma_start(wo_sb[:, g, g2 * (H // 2):(g2 + 1) * (H // 2)],
                                  wo_r[:, g, g2 * (H // 2):(g2 + 1) * (H // 2)])
        nc.sync.dma_start(tri_sb[:], tri)

        # identity (bf16) for PE transposes of V
        ident = pers.tile([P, P], BF16, tag="ident")
        nc.gpsimd.memset(ident[:], 1.0)
        nc.gpsimd.affine_select(
            out=ident[:], in_=ident[:],
            compare_op=mybir.AluOpType.is_equal, fill=0.0,
            base=0, pattern=[[-1, P]], channel_multiplier=1,
        )

        qt_sb = [pers.tile([P, NQ], BF16, tag=f"qt{c}", name=f"qt{c}")
                 for c in range(NCC)]
        kt_sb = pers.tile([P, NQ], BF16, tag="kt")          # [K_T ; K_T]
        v_sb = pers.tile([P, NQ // KC, HD + 1], BF16, tag="v")      # [V|1]
        nc.gpsimd.memset(v_sb[:, :, HD], 1.0)
        ctx_sb = pers.tile([P, 2, NCC, QT], BF16, tag="ctx")

        # ---- SBUF pools ----
        hpool = ctx.enter_context(tc.tile_pool(name="hpool", bufs=2))
        vt_pool = ctx.enter_context(tc.tile_pool(name="vtp", bufs=2))
        pt_pool = ctx.enter_context(tc.tile_pool(name="ptp", bufs=4))
        dpool = ctx.enter_context(tc.tile_pool(name="dpool", bufs=3))
        bcpool = ctx.enter_context(tc.tile_pool(name="bcp", bufs=2))
        obpool = ctx.enter_context(tc.tile_pool(name="obp", bufs=3))
        if mask_mode == "full":
            mpool = ctx.enter_context(tc.tile_pool(name="mpool", bufs=4))

        # ---- PSUM pool (tags: pq 1, pkv 1, sps 2, cps 2, pow 2) ----
        psum = ctx.enter_context(tc.tile_pool(name="psum", bufs=1, space="PSUM"))

        ht_tiles = {}                   # blk -> [4 x tile [128,4,512]]

        # ================= stream generators =================
        # Each stream yields (kind, closure); emission interleaves streams.

        def prefetch_stream(blk):
            """Issue the 4 coarse ht DMAs for superblock blk."""
            g0 = blk * QT
            tiles = []
            for g in range(4):
                h4 = hpool.tile([P, 4, QT], BF16, tag=f"h{g}", name=f"h4_{g}")
                tiles.append(h4)
            ht_tiles[blk] = tiles

            def mk(g):
                def emit():
                    nc.sync.dma_start(tiles[g][:],
                                      ht_r[:, g * 4:(g + 1) * 4, g0:g0 + QT])
                return emit
            for g in range(4):
                yield ('dma', mk(g))

        def proj_stream(blk):
            """Projections of superblock blk -> qt_sb/kt_sb/v_sb columns."""
            g0 = blk * QT
            hts = ht_tiles[blk]
            pq0 = psum.tile([P, QT], F32, tag="pq0", bufs=1, name="pq0")
            pq1 = psum.tile([P, QT], F32, tag="pq1", bufs=1, name="pq1")
            pkv = psum.tile([P, QT], F32, tag="pkv", bufs=1, name="pkv")

            def mk_mm(hc, pq0=pq0, pq1=pq1, pkv=pkv):
                def emit():
                    mv = hts[hc // 4][:, hc % 4, :]
                    fl = dict(start=(hc == 0), stop=(hc == NHC - 1))
                    nc.tensor.matmul(pq0[:], wq_sb[:, hc, 0:P], mv, **fl)
                    nc.tensor.matmul(pq1[:], wq_sb[:, hc, P:CL], mv, **fl)
                    nc.tensor.matmul(pkv[:], wkv_sb[:, hc, :], mv, **fl)
                return emit
            for hc in range(NHC):
                yield ('mm3', mk_mm(hc))

            vtmp = vt_pool.tile([P, QT], BF16, tag="vt", name="vtmp")

            def drain(pq0=pq0, pq1=pq1, pkv=pkv, vtmp=vtmp):
                nc.vector.tensor_copy(qt_sb[0][:, g0:g0 + QT], pq0[:])
                nc.vector.tensor_copy(qt_sb[1][:, g0:g0 + QT], pq1[:])
                nc.vector.tensor_copy(kt_sb[:HD, g0:g0 + QT], pkv[:HD, :])
                nc.vector.tensor_copy(vtmp[HD:2 * HD, :], pkv[HD:2 * HD, :])
            yield ('drain', drain)

            def mk_tr(j, vtmp=vtmp):
                kcg = g0 // KC + j

                def emit():
                    tp = psum.tile([P, HD], BF16, tag="sps", bufs=2, name="tp")
                    nc.tensor.transpose(
                        tp[:, :HD],
                        vtmp[HD:2 * HD, j * KC:(j + 1) * KC],
                        ident[HD:2 * HD, HD:2 * HD],
                    )
                    nc.scalar.activation(v_sb[:, kcg, :HD], tp[:, :HD], CPY)
                return emit
            for j in range(DPT):
                yield ('tr', mk_tr(j))
            # duplicate K rows 0-63 -> 64-127 for odd heads
            yield ('dma', lambda: nc.gpsimd.dma_start(
                kt_sb[HD:2 * HD, g0:g0 + QT], kt_sb[:HD, g0:g0 + QT]))

        def attn_stream(blk):
            """Attention of superblock blk into ctx_sb[:, blk%2]."""
            b, qtb = blk // QPB, blk % QPB
            g0 = blk * QT
            ib = blk % 2
            nkc = (qtb + 1) * DPT if mask_mode == "causal" else KPB
            for h in range(NHL):
                hb = (h % 2) * HD
                cc = h // 2
                even = (h % 2 == 0)
                cps = psum.tile([P, QT], F32, tag="cps", bufs=2, name="cps")
                pend = []   # deferred AV emissions (one-unit lag)

                def emit_av(item, cps=cps, nkc=nkc):
                    akc, alo, apt, akcg = item
                    nc.tensor.matmul(cps[0:HD + 1, alo:], v_sb[:, akcg, :],
                                     apt[:, alo:],
                                     start=(akc == 0), stop=(akc == nkc - 1))

                def mk_unit(kc, hb=hb, cc=cc, pend=pend, emit_av=emit_av, b=b):
                    kcg = b * KPB + kc
                    do = kc * KC - qtb * QT if mask_mode == "causal" else -1
                    lo = max(do, 0)

                    def emit():
                        sps = psum.tile([P, QT], F32, tag="sps", bufs=2,
                                        name="sps")
                        nc.tensor.matmul(
                            sps[:, lo:],
                            kt_sb[hb:hb + HD, kcg * KC:(kcg + 1) * KC],
                            qt_sb[cc][hb:hb + HD, g0 + lo:g0 + QT],
                            start=True, stop=True,
                        )
                        if pend:
                            emit_av(pend.pop(0))
                        if mask_mode == "full":
                            mt = mpool.tile([KC, QT], F32, tag="mt", name="mt")
                            nc.sync.dma_start(
                                mt[:], maskt[b, kc * KC:(kc + 1) * KC,
                                             (g0 - b * S):(g0 - b * S) + QT])
                            nc.vector.tensor_add(sps[:], sps[:], mt[:])
                        elif do >= 0:
                            nc.vector.tensor_add(
                                sps[:, do:do + KC], sps[:, do:do + KC], tri_sb[:])
                        pt = pt_pool.tile([P, QT], BF16, tag="pt", name="pt")
                        nc.scalar.activation(pt[:, lo:], sps[:, lo:], EXP)
                        pend.append((kc, lo, pt, kcg))
                    return emit

                for kc in range(nkc):
                    yield ('attn', mk_unit(kc))

                def finalize(h=h, cc=cc, even=even, cps=cps, pend=pend,
                             emit_av=emit_av, ib=ib, blk=blk):
                    while pend:
                        emit_av(pend.pop(0))
                    # normalize: denom row -> dram -> [4,128] -> recip -> dram
                    # -> [1,512] -> broadcast.  (SBUF partition-reshape DMAs
                    # are illegal; the DRAM bounce is the legal spelling.)
                    hh = blk * NHL + h
                    den = dpool.tile([P, QT], F32, tag="den", name="den")
                    nc.scalar.activation(den[HD:HD + 1, :],
                                         cps[HD:HD + 1, :], CPY)
                    nc.gpsimd.dma_start(dscr[hh:hh + 1, :], den[HD:HD + 1, :])
                    dh = dpool.tile([4, KC], F32, tag="dh", name="dh")
                    nc.gpsimd.dma_start(
                        dh[:],
                        dscr[hh:hh + 1, :].rearrange("o (a b) -> (o a) b", a=4))
                    rc = dpool.tile([4, KC], F32, tag="rc", name="rc")
                    nc.vector.reciprocal(rc[:], dh[:])
                    nc.gpsimd.dma_start(
                        dscr2[hh:hh + 1, :].rearrange("o (a b) -> (o a) b", a=4),
                        rc[:])
                    rb = dpool.tile([1, QT], F32, tag="rb", name="rb")
                    nc.gpsimd.dma_start(rb[:], dscr2[hh:hh + 1, :])
                    if debug_dump:
                        nc.sync.dma_start(dbg_rb[hh:hh + 1, :], rb[:])
                    bc = bcpool.tile([P, QT], F32, tag="bc", name="bc")
                    nc.gpsimd.partition_broadcast(bc[0:HD, :], rb[:])
                    if even:
                        nc.vector.tensor_mul(ctx_sb[0:HD, ib, cc, :],
                                             cps[0:HD, :], bc[0:HD, :])
                    else:
                        ctmp = bcpool.tile([HD, QT], BF16, tag="ctmp",
                                           name="ctmp")
                        nc.vector.tensor_mul(ctmp[:], cps[0:HD, :], bc[0:HD, :])
                        nc.gpsimd.dma_start(ctx_sb[HD:2 * HD, ib, cc, :],
                                            ctmp[:])
                yield ('fin', finalize)

        def wo_stream(blk):
            """Output projection of superblock blk from ctx_sb[:, blk%2]."""
            ib = blk % 2
            r0 = blk * QT
            ET = 512
            last_blk = (blk == NBLK - 1)
            if debug_dump:
                yield ('dbg', lambda: nc.sync.dma_start(
                    dbg_ctx[blk], ctx_sb[:, ib, :, :]))
            for qc in range(QT // P):
                ob = obpool.tile([P, H], BF16, tag="ob", name="ob")

                def mk_unit(et, qc=qc, ob=ob):
                    def emit():
                        if last_blk:
                            # attention is done: cps/sps banks are free, so
                            # rotate through 3 banks to avoid drain stalls
                            tg = ["pow", "cps", "sps"][(qc * 4 + et) % 3]
                            bufs = {"pow": 1, "cps": 2, "sps": 2}[tg]
                        else:
                            tg, bufs = "pow", 1
                        po = psum.tile([P, ET], F32, tag=tg, bufs=bufs,
                                       name="po")
                        for cc2 in range(NCC):
                            nc.tensor.matmul(
                                po[:],
                                ctx_sb[:, ib, cc2, qc * P:(qc + 1) * P],
                                wo_sb[:, cc2, et * ET:(et + 1) * ET],
                                start=(cc2 == 0), stop=(cc2 == NCC - 1),
                            )
                        dst = ob[:, et * ET:(et + 1) * ET]
                        nc.vector.tensor_copy(dst, po[:])
                        if last_blk:
                            nc.gpsimd.dma_start(
                                out_p[r0 + qc * P:r0 + (qc + 1) * P,
                                      et * ET:(et + 1) * ET], dst)
                        elif et % 2 == 1:
                            nc.gpsimd.dma_start(
                                out_p[r0 + qc * P:r0 + (qc + 1) * P,
                                      (et - 1) * ET:(et + 1) * ET],
                                ob[:, (et - 1) * ET:(et + 1) * ET])
                    return emit
                for et in range(H // ET):
                    yield ('wo', mk_unit(et))

        # ================= merge + emit =================
        def merge(streams):
            """Proportional interleave of unit streams (virtual-time merge)."""
            lists = [list(s) for s in streams if s is not None]
            lists = [l for l in lists if l]
            idx = [0] * len(lists)
            while True:
                best, bestv = -1, 2.0
                for j, l in enumerate(lists):
                    if idx[j] < len(l):
                        v = (idx[j] + 0.5) / len(l)
                        if v < bestv:
                            best, bestv = j, v
                if best < 0:
                    break
                lists[best][idx[best]][1]()
                idx[best] += 1

        # prologue: fine-grained ht loads for blocks 0-1 so the first
        # matmuls start early, then projections of blocks 0 and 1.
        for blk0 in range(2):
            tiles0 = [hpool.tile([P, 4, QT], BF16, tag=f"h{g}", name=f"h4p_{g}")
                      for g in range(4)]
            ht_tiles[blk0] = tiles0
            for g in range(4):
                for j in range(4):
                    nc.sync.dma_start(tiles0[g][:, j, :],
                                      ht_r[:, g * 4 + j, blk0 * QT:(blk0 + 1) * QT])
        for _, emit in proj_stream(0):
            emit()
        # main loop: attention trails projections by one slot
        for i in range(NBLK):
            streams = [attn_stream(i)]
            if i + 1 < NBLK:
                streams.append(proj_stream(i + 1))
            if i + 2 < NBLK:
                streams.append(prefetch_stream(i + 2))
            if i - 1 >= 0:
                streams.append(wo_stream(i - 1))
            merge(streams)
        # epilogue
        for _, emit in wo_stream(NBLK - 1):
            emit()
        if debug_dump:
            for c in range(NCC):
                nc.sync.dma_start(dbg_qt[c], qt_sb[c][:])
            nc.sync.dma_start(dbg_kt[:], kt_sb[:])
            nc.sync.dma_start(dbg_v[:], v_sb[:])


    nc.compile()
    return nc


def _detect_mask_mode(m, S):
    if not np.any(m):
        return "zeros"
    b0 = np.asarray(m[0, 0])
    qi = np.arange(S)
    tl = qi[None, :] <= qi[:, None]
    if (b0[tl] == 0.0).all() and (b0[~tl] <= -1e8).all() and (m == b0).all():
        return "causal"
    return "full"


def shard_inputs(hidden_states, attention_mask, Wq, Wk, Wv, Wo, mask_mode):
    import ml_dtypes
    bf16 = ml_dtypes.bfloat16
    B, S, H = hidden_states.shape
    NH = Wq.shape[1] // HD
    NKV = Wk.shape[1] // HD
    NHL = NH // N_CORES
    scale = np.float32(1.0 / np.sqrt(HD))

    ht = np.ascontiguousarray(
        hidden_states.reshape(B * S, H).T).astype(bf16)
    ki = np.arange(KC)
    tri = np.where(ki[:, None] <= ki[None, :], 0.0, NEG).astype(np.float32)
    if mask_mode == "full":
        maskt = np.ascontiguousarray(
            np.asarray(attention_mask)[:, 0].transpose(0, 2, 1).astype(np.float32))

    in_maps = []
    for c in range(N_CORES):
        wq_c = np.ascontiguousarray(
            Wq[:, c * NHL * HD:(c + 1) * NHL * HD] * scale).astype(bf16)
        kv0 = c * (NKV // N_CORES) * HD
        wkv_c = np.ascontiguousarray(np.concatenate(
            [Wk[:, kv0:kv0 + HD], Wv[:, kv0:kv0 + HD]], axis=1)).astype(bf16)
        wo_c = np.ascontiguousarray(
            Wo[c * NHL * HD:(c + 1) * NHL * HD, :]).astype(bf16)
        im = {"ht": ht, "wq": wq_c, "wkv": wkv_c, "wo": wo_c, "tri": tri}
        if mask_mode == "full":
            im["maskt"] = maskt
        in_maps.append(im)
    return in_maps, NHL


def kernel(hidden_states, attention_mask, Wq, Wk, Wv, Wo):
    global LAST_RESULT
    hidden_states = np.asarray(hidden_states, dtype=np.float32)
    attention_mask = np.asarray(attention_mask, dtype=np.float32)
    Wq, Wk, Wv, Wo = (np.asarray(w, dtype=np.float32) for w in (Wq, Wk, Wv, Wo))
    B, S, H = hidden_states.shape

    mask_mode = _detect_mask_mode(attention_mask, S)
    in_maps, NHL = shard_inputs(hidden_states, attention_mask, Wq, Wk, Wv, Wo,
                                mask_mode)

    key = (B, S, H, NHL, mask_mode)
    if key not in _nc_cache:
        _nc_cache[key] = build_attn_core(B=B, S=S, H=H, NHL=NHL,
                                         mask_mode=mask_mode)
    nc = _nc_cache[key]

    res = run_bass_kernel_spmd(nc, in_maps, core_ids=list(range(N_CORES)),
                               trace=TRACE, trace_cores=TRACE_CORES)
    LAST_RESULT = res

    out = res.results[0]["out_p"].astype(np.float32)
    for c in range(1, N_CORES):
        out = out + res.results[c]["out_p"].astype(np.float32)
    return out.reshape(B, S, H)


# revision 16
# speedup vs baseline: 1.0627x; 1.0312x over previous
"""GQA causal attention (B=2, S=2048, H=2048, 32 Q heads / 8 KV heads, hd=64)
as an 8-way tensor-parallel Trainium2 Bass kernel.

Sharding: heads. Each NeuronCore gets 4 Q heads + their KV head (Wq/Wk/Wv
column slices, Wo row slice), computes a partial output over the full batch,
and the host sums the 8 bf16 partials (the Wo all-reduce done host-side).

v2 design (vs the fp32r baseline): everything bf16 on the PE, and the whole
kernel is ONE software-pipelined loop over 8 superblocks of 512 query
positions.  In slot i the instruction stream interleaves four stages --
ht prefetch for block i+2, projections of block i+1, attention of block i,
output projection of block i-1 -- so the tensor engine always has an
independent matmul ready and stays at its full (ramped) clock.  Causal
structure is exploited at matmul granularity: scores/exp/AV only touch
columns q >= k.

Per-core dataflow (d-major, no activation transposes except V):
    Q_T  = (Wq_c * scale)^T @ hidden^T          [256, B*S]   (heads stacked)
    K_T  = Wk_c^T @ hidden^T  (rows 0-63, duplicated to 64-127 for odd heads)
    V    = PE-transpose(Wv_c^T @ hidden^T)      [keys, 64] stored [V|1|V]
    S_T[k,q] = K_T(chunk)^T x Q_T               causal chunks only
    P_T  = exp(S_T + tri on diagonal chunks)    bf16
    ctx_aug = [V|1]^T @ P_T                     even heads -> psum rows 0-64
              [1|V]^T @ P_T                     odd heads  -> psum rows 63-127
    ctx  = ctx_aug * bcast(1/denom)             denom recip via [4,128] DVE
    out_partial = ctx^T @ Wo_c                  [B*S, 2048] bf16
"""

import sys

for _p in ("/root/.axon_site", "/root/.axon_site/_ro/trn_rl_repo",
           "/root/.axon_site/_ro/pypackages", "/opt/trn_rl_repo", "/opt/pypackages"):
    if _p not in sys.path:
        sys.path.append(_p)

from contextlib import ExitStack

import numpy as np

import concourse.bass as bass  # noqa: F401
import concourse.tile as tile
from concourse import bacc, mybir
from concourse.bass_utils import run_bass_kernel_spmd

F32 = mybir.dt.float32
BF16 = mybir.dt.bfloat16
P = 128
KC = 128
QT = 512
N_CORES = 8
HD = 64
NEG = -1e9

TRACE = False            # test harness flips this for NTFF profiling
TRACE_CORES = None
LAST_RESULT = None       # BassKernelResults of the last run (for the harness)

_nc_cache = {}


def build_attn_core(B=2, S=2048, H=2048, NHL=4, mask_mode="causal", debug_dump=False):
    """Build + bass-compile the per-core program.

    DRAM inputs (per core):
      ht  [H, B*S] bf16   hidden transposed      wq [H, NHL*HD] bf16 (pre-scaled)
      wkv [H, 2*HD] bf16  [Wk_c | Wv_c]          wo [NHL*HD, H] bf16
      tri [KC, KC] f32    transposed causal block mask (tri[k,q]=0 iff k<=q)
      maskt [B, S, S] f32 (only mask_mode=="full") additive mask transposed
    Output: out_p [B*S, H] bf16.
    """
    NQ = B * S
    CL = NHL * HD                       # 256 q-head cols per core
    assert H % P == 0 and S % QT == 0
    NHC = H // P                        # 16 contraction chunks
    NCC = CL // P                       # 2 head-pair groups
    QPB = S // QT                       # 4 q-blocks per batch
    NBLK = B * QPB                      # 8 superblocks
    KPB = S // KC                       # 16 key chunks per batch
    DPT = QT // KC                      # 4 key chunks per q-block
    EXP = mybir.ActivationFunctionType.Exp
    CPY = mybir.ActivationFunctionType.Copy

    nc = bacc.Bacc("TRN2", target_bir_lowering=False, debug=False)

    ht = nc.dram_tensor("ht", [H, NQ], BF16, kind="ExternalInput").ap()
    wq = nc.dram_tensor("wq", [H, CL], BF16, kind="ExternalInput").ap()
    wkv = nc.dram_tensor("wkv", [H, 2 * HD], BF16, kind="ExternalInput").ap()
    wo = nc.dram_tensor("wo", [CL, H], BF16, kind="ExternalInput").ap()
    tri = nc.dram_tensor("tri", [KC, KC], F32, kind="ExternalInput").ap()
    if mask_mode == "full":
        maskt = nc.dram_tensor("maskt", [B, S, S], F32, kind="ExternalInput").ap()
    out_p = nc.dram_tensor("out_p", [NQ, H], BF16, kind="ExternalOutput").ap()
    dscr = nc.dram_tensor("dscr", [NBLK * NHL, QT], F32, kind="Internal").ap()
    if debug_dump:
        dbg_qt = nc.dram_tensor("dbg_qt", [NCC, P, NQ], BF16, kind="ExternalOutput").ap()
        dbg_kt = nc.dram_tensor("dbg_kt", [P, NQ], BF16, kind="ExternalOutput").ap()
        dbg_v = nc.dram_tensor("dbg_v", [P, NQ // KC, HD + 1], BF16, kind="ExternalOutput").ap()
        dbg_ctx = nc.dram_tensor("dbg_ctx", [NBLK, P, NCC, QT], BF16, kind="ExternalOutput").ap()
        dbg_rb = nc.dram_tensor("dbg_rb", [NBLK * NHL, QT], F32, kind="ExternalOutput").ap()
    dscr2 = nc.dram_tensor("dscr2", [NBLK * NHL, QT], F32, kind="Internal").ap()

    ht_r = ht.rearrange("(o p) m -> p o m", p=P)      # [128, 16, 4096]

    with tile.TileContext(nc) as tc, ExitStack() as ctx:
        # ---- persistent SBUF ----
        pers = ctx.enter_context(tc.tile_pool(name="pers", bufs=1))
        wq_sb = pers.tile([P, NHC, CL], BF16, tag="wq")
        wkv_sb = pers.tile([P, NHC, 2 * HD], BF16, tag="wkv")
        wo_sb = pers.tile([P, NCC, H], BF16, tag="wo")
        tri_sb = pers.tile([KC, KC], F32, tag="tri")
        # weight loads split so the first projection can start early
        wq_r = wq.rearrange("(o p) m -> p o m", p=P)
        wkv_r = wkv.rearrange("(o p) m -> p o m", p=P)
        wo_r = wo.rearrange("(o p) m -> p o m", p=P)
        for g in range(8):
            nc.sync.dma_start(wq_sb[:, g * 2:(g + 1) * 2, :],
                              wq_r[:, g * 2:(g + 1) * 2, :])
        for g in range(2):
            nc.sync.dma_start(wkv_sb[:, g * 8:(g + 1) * 8, :],
                              wkv_r[:, g * 8:(g + 1) * 8, :])
        for g in range(2):
            for g2 in range(2):
                nc.sync.dma_start(wo_sb[:, g, g2 * (H // 2):(g2 + 1) * (H // 2)],
                                  wo_r[:, g, g2 * (H // 2):(g2 + 1) * (H // 2)])
        nc.sync.dma_start(tri_sb[:], tri)

        # identity (bf16) for PE transposes of V
        ident = pers.tile([P, P], BF16, tag="ident")
        nc.gpsimd.memset(ident[:], 1.0)
        nc.gpsimd.affine_select(
            out=ident[:], in_=ident[:],
            compare_op=mybir.AluOpType.is_equal, fill=0.0,
            base=0, pattern=[[-1, P]], channel_multiplier=1,
        )

        qt_sb = [pers.tile([P, NQ], BF16, tag=f"qt{c}", name=f"qt{c}")
                 for c in range(NCC)]
        kt_sb = pers.tile([P, NQ], BF16, tag="kt")          # [K_T ; K_T]
        v_sb = pers.tile([P, NQ // KC, HD + 1], BF16, tag="v")      # [V|1]
        nc.gpsimd.memset(v_sb[:, :, HD], 1.0)
        ctx_sb = pers.tile([P, 2, NCC, QT], BF16, tag="ctx")

        # ---- SBUF pools ----
        hpool = ctx.enter_context(tc.tile_pool(name="hpool", bufs=2))
        vt_pool = ctx.enter_context(tc.tile_pool(name="vtp", bufs=2))
        pt_pool = ctx.enter_context(tc.tile_pool(name="ptp", bufs=4))
        dpool = ctx.enter_context(tc.tile_pool(name="dpool", bufs=3))
        bcpool = ctx.enter_context(tc.tile_pool(name="bcp", bufs=2))
        obpool = ctx.enter_context(tc.tile_pool(name="obp", bufs=3))
        if mask_mode == "full":
            mpool = ctx.enter_context(tc.tile_pool(name="mpool", bufs=4))

        # ---- PSUM pool (tags: pq 1, pkv 1, sps 2, cps 2, pow 2) ----
        psum = ctx.enter_context(tc.tile_pool(name="psum", bufs=1, space="PSUM"))

        ht_tiles = {}                   # blk -> [4 x tile [128,4,512]]

        # ================= stream generators =================
        # Each stream yields (kind, closure); emission interleaves streams.

        def prefetch_stream(blk):
            """Issue the 4 coarse ht DMAs for superblock blk."""
            g0 = blk * QT
            tiles = []
            for g in range(4):
                h4 = hpool.tile([P, 4, QT], BF16, tag=f"h{g}", name=f"h4_{g}")
                tiles.append(h4)
            ht_tiles[blk] = tiles

            def mk(g):
                def emit():
                    nc.sync.dma_start(tiles[g][:],
                                      ht_r[:, g * 4:(g + 1) * 4, g0:g0 + QT])
                return emit
            for g in range(4):
                yield ('dma', mk(g))

        def proj_stream(blk):
            """Projections of superblock blk -> qt_sb/kt_sb/v_sb columns."""
            g0 = blk * QT
            hts = ht_tiles[blk]
            pq0 = psum.tile([P, QT], F32, tag="pq0", bufs=1, name="pq0")
            pq1 = psum.tile([P, QT], F32, tag="pq1", bufs=1, name="pq1")
            pkv = psum.tile([P, QT], F32, tag="pkv", bufs=1, name="pkv")

            def mk_mm(hc, pq0=pq0, pq1=pq1, pkv=pkv):
                def emit():
                    mv = hts[hc // 4][:, hc % 4, :]
                    fl = dict(start=(hc == 0), stop=(hc == NHC - 1))
                    nc.tensor.matmul(pq0[:], wq_sb[:, hc, 0:P], mv, **fl)
                    nc.tensor.matmul(pq1[:], wq_sb[:, hc, P:CL], mv, **fl)
                    nc.tensor.matmul(pkv[:], wkv_sb[:, hc, :], mv, **fl)
                return emit
            for hc in range(NHC):
                yield ('mm3', mk_mm(hc))

            vtmp = vt_pool.tile([P, QT], BF16, tag="vt", name="vtmp")

            def drain(pq0=pq0, pq1=pq1, pkv=pkv, vtmp=vtmp):
                nc.vector.tensor_copy(qt_sb[0][:, g0:g0 + QT], pq0[:])
                nc.vector.tensor_copy(qt_sb[1][:, g0:g0 + QT], pq1[:])
                nc.vector.tensor_copy(kt_sb[:HD, g0:g0 + QT], pkv[:HD, :])
                nc.vector.tensor_copy(vtmp[HD:2 * HD, :], pkv[HD:2 * HD, :])
            yield ('drain', drain)

            def mk_tr(j, vtmp=vtmp):
                kcg = g0 // KC + j

                def emit():
                    tp = psum.tile([P, HD], BF16, tag="pkv", bufs=1, name="tp")
                    nc.tensor.transpose(
                        tp[:, :HD],
                        vtmp[HD:2 * HD, j * KC:(j + 1) * KC],
                        ident[HD:2 * HD, HD:2 * HD],
                    )
                    nc.vector.tensor_copy(v_sb[:, kcg, :HD], tp[:, :HD])
                return emit
            for j in range(DPT):
                yield ('tr', mk_tr(j))
            # duplicate K rows 0-63 -> 64-127 for odd heads
            yield ('dma', lambda: nc.gpsimd.dma_start(
                kt_sb[HD:2 * HD, g0:g0 + QT], kt_sb[:HD, g0:g0 + QT]))

        def attn_stream(blk):
            """Attention of superblock blk into ctx_sb[:, blk%2]."""
            b, qtb = blk // QPB, blk % QPB
            g0 = blk * QT
            ib = blk % 2
            nkc = (qtb + 1) * DPT if mask_mode == "causal" else KPB
            for h in range(NHL):
                hb = (h % 2) * HD
                cc = h // 2
                even = (h % 2 == 0)
                cps = psum.tile([P, QT], F32, tag="cps", bufs=2, name="cps")
                pend = []   # deferred AV emissions (one-unit lag)

                def emit_av(item, cps=cps, nkc=nkc):
                    akc, alo, apt, akcg = item
                    nc.tensor.matmul(cps[0:HD + 1, alo:], v_sb[:, akcg, :],
                                     apt[:, alo:],
                                     start=(akc == 0), stop=(akc == nkc - 1))

                def mk_unit(kc, hb=hb, cc=cc, pend=pend, emit_av=emit_av, b=b):
                    kcg = b * KPB + kc
                    do = kc * KC - qtb * QT if mask_mode == "causal" else -1
                    lo = max(do, 0)

                    def emit():
                        sps = psum.tile([P, QT], F32, tag="sps", bufs=2,
                                        name="sps")
                        nc.tensor.matmul(
                            sps[:, lo:],
                            kt_sb[hb:hb + HD, kcg * KC:(kcg + 1) * KC],
                            qt_sb[cc][hb:hb + HD, g0 + lo:g0 + QT],
                            start=True, stop=True,
                        )
                        if pend:
                            emit_av(pend.pop(0))
                        if mask_mode == "full":
                            mt = mpool.tile([KC, QT], F32, tag="mt", name="mt")
                            nc.sync.dma_start(
                                mt[:], maskt[b, kc * KC:(kc + 1) * KC,
                                             (g0 - b * S):(g0 - b * S) + QT])
                            nc.vector.tensor_add(sps[:], sps[:], mt[:])
                        elif do >= 0:
                            nc.vector.tensor_add(
                                sps[:, do:do + KC], sps[:, do:do + KC], tri_sb[:])
                        pt = pt_pool.tile([P, QT], BF16, tag="pt", name="pt")
                        nc.scalar.activation(pt[:, lo:], sps[:, lo:], EXP)
                        pend.append((kc, lo, pt, kcg))
                    return emit

                for kc in range(nkc):
                    yield ('attn', mk_unit(kc))

                def finalize(h=h, cc=cc, even=even, cps=cps, pend=pend,
                             emit_av=emit_av, ib=ib, blk=blk):
                    while pend:
                        emit_av(pend.pop(0))
                    # normalize: denom row -> dram -> [4,128] -> recip -> dram
                    # -> [1,512] -> broadcast.  (SBUF partition-reshape DMAs
                    # are illegal; the DRAM bounce is the legal spelling.)
                    hh = blk * NHL + h
                    den = dpool.tile([P, QT], F32, tag="den", name="den")
                    nc.scalar.activation(den[HD:HD + 1, :],
                                         cps[HD:HD + 1, :], CPY)
                    nc.gpsimd.dma_start(dscr[hh:hh + 1, :], den[HD:HD + 1, :])
                    dh = dpool.tile([4, KC], F32, tag="dh", name="dh")
                    nc.gpsimd.dma_start(
                        dh[:],
                        dscr[hh:hh + 1, :].rearrange("o (a b) -> (o a) b", a=4))
                    rc = dpool.tile([4, KC], F32, tag="rc", name="rc")
                    nc.vector.reciprocal(rc[:], dh[:])
                    nc.gpsimd.dma_start(
                        dscr2[hh:hh + 1, :].rearrange("o (a b) -> (o a) b", a=4),
                        rc[:])
                    rb = dpool.tile([1, QT], F32, tag="rb", name="rb")
                    nc.gpsimd.dma_start(rb[:], dscr2[hh:hh + 1, :])
                    if debug_dump:
                        nc.sync.dma_start(dbg_rb[hh:hh + 1, :], rb[:])
                    bc = bcpool.tile([P, QT], F32, tag="bc", name="bc")
                    nc.gpsimd.partition_broadcast(bc[0:HD, :], rb[:])
                    if even:
                        nc.vector.tensor_mul(ctx_sb[0:HD, ib, cc, :],
                                             cps[0:HD, :], bc[0:HD, :])
                    else:
                        ctmp = bcpool.tile([HD, QT], BF16, tag="ctmp",
                                           name="ctmp")
                        nc.vector.tensor_mul(ctmp[:], cps[0:HD, :], bc[0:HD, :])
                        nc.gpsimd.dma_start(ctx_sb[HD:2 * HD, ib, cc, :],
                                            ctmp[:])
                yield ('fin', finalize)

        def wo_stream(blk):
            """Output projection of superblock blk from ctx_sb[:, blk%2]."""
            ib = blk % 2
            r0 = blk * QT
            ET = 512
            last_blk = (blk == NBLK - 1)
            if debug_dump:
                yield ('dbg', lambda: nc.sync.dma_start(
                    dbg_ctx[blk], ctx_sb[:, ib, :, :]))
            for qc in range(QT // P):
                ob = obpool.tile([P, H], BF16, tag="ob", name="ob")

                def mk_unit(et, qc=qc, ob=ob):
                    def emit():
                        if last_blk:
                            # attention is done: cps/sps banks are free, so
                            # rotate through 3 banks to avoid drain stalls
                            tg = ["pow", "cps", "sps"][(qc * 4 + et) % 3]
                            bufs = {"pow": 1, "cps": 2, "sps": 2}[tg]
                        else:
                            tg, bufs = "pow", 1
                        po = psum.tile([P, ET], F32, tag=tg, bufs=bufs,
                                       name="po")
                        for cc2 in range(NCC):
                            nc.tensor.matmul(
                                po[:],
                                ctx_sb[:, ib, cc2, qc * P:(qc + 1) * P],
                                wo_sb[:, cc2, et * ET:(et + 1) * ET],
                                start=(cc2 == 0), stop=(cc2 == NCC - 1),
                            )
                        dst = ob[:, et * ET:(et + 1) * ET]
                        nc.vector.tensor_copy(dst, po[:])
                        if last_blk:
                            nc.gpsimd.dma_start(
                                out_p[r0 + qc * P:r0 + (qc + 1) * P,
                                      et * ET:(et + 1) * ET], dst)
                        elif et % 2 == 1:
                            nc.gpsimd.dma_start(
                                out_p[r0 + qc * P:r0 + (qc + 1) * P,
                                      (et - 1) * ET:(et + 1) * ET],
                                ob[:, (et - 1) * ET:(et + 1) * ET])
                    return emit
                for et in range(H // ET):
                    yield ('wo', mk_unit(et))

        # ================= merge + emit =================
        def merge(streams):
            """Proportional interleave of unit streams (virtual-time merge)."""
            lists = [list(s) for s in streams if s is not None]
            lists = [l for l in lists if l]
            idx = [0] * len(lists)
            while True:
                best, bestv = -1, 2.0
                for j, l in enumerate(lists):
                    if idx[j] < len(l):
                        v = (idx[j] + 0.5) / len(l)
                        if v < bestv:
                            best, bestv = j, v
                if best < 0:
                    break
                lists[best][idx[best]][1]()
                idx[best] += 1

        # prologue: fine-grained ht loads for blocks 0-1 so the first
        # matmuls start early, then projections of blocks 0 and 1.
        for blk0 in range(2):
            tiles0 = [hpool.tile([P, 4, QT], BF16, tag=f"h{g}", name=f"h4p_{g}")
                      for g in range(4)]
            ht_tiles[blk0] = tiles0
            for g in range(4):
                for j in range(4):
                    nc.sync.dma_start(tiles0[g][:, j, :],
                                      ht_r[:, g * 4 + j, blk0 * QT:(blk0 + 1) * QT])
        for _, emit in proj_stream(0):
            emit()
        # main loop: attention trails projections by one slot
        for i in range(NBLK):
            streams = [attn_stream(i)]
            if i + 1 < NBLK:
                streams.append(proj_stream(i + 1))
            if i + 2 < NBLK:
                streams.append(prefetch_stream(i + 2))
            if i - 1 >= 0:
                streams.append(wo_stream(i - 1))
            merge(streams)
        # epilogue
        for _, emit in wo_stream(NBLK - 1):
            emit()
        if debug_dump:
            for c in range(NCC):
                nc.sync.dma_start(dbg_qt[c], qt_sb[c][:])
            nc.sync.dma_start(dbg_kt[:], kt_sb[:])
            nc.sync.dma_start(dbg_v[:], v_sb[:])


    nc.compile()
    return nc


def _detect_mask_mode(m, S):
    if not np.any(m):
        return "zeros"
    b0 = np.asarray(m[0, 0])
    qi = np.arange(S)
    tl = qi[None, :] <= qi[:, None]
    if (b0[tl] == 0.0).all() and (b0[~tl] <= -1e8).all() and (m == b0).all():
        return "causal"
    return "full"


def shard_inputs(hidden_states, attention_mask, Wq, Wk, Wv, Wo, mask_mode):
    import ml_dtypes
    bf16 = ml_dtypes.bfloat16
    B, S, H = hidden_states.shape
    NH = Wq.shape[1] // HD
    NKV = Wk.shape[1] // HD
    NHL = NH // N_CORES
    scale = np.float32(1.0 / np.sqrt(HD))

    ht = np.ascontiguousarray(
        hidden_states.reshape(B * S, H).T).astype(bf16)
    ki = np.arange(KC)
    tri = np.where(ki[:, None] <= ki[None, :], 0.0, NEG).astype(np.float32)
    if mask_mode == "full":
        maskt = np.ascontiguousarray(
            np.asarray(attention_mask)[:, 0].transpose(0, 2, 1).astype(np.float32))

    in_maps = []
    for c in range(N_CORES):
        wq_c = np.ascontiguousarray(
            Wq[:, c * NHL * HD:(c + 1) * NHL * HD] * scale).astype(bf16)
        kv0 = c * (NKV // N_CORES) * HD
        wkv_c = np.ascontiguousarray(np.concatenate(
            [Wk[:, kv0:kv0 + HD], Wv[:, kv0:kv0 + HD]], axis=1)).astype(bf16)
        wo_c = np.ascontiguousarray(
            Wo[c * NHL * HD:(c + 1) * NHL * HD, :]).astype(bf16)
        im = {"ht": ht, "wq": wq_c, "wkv": wkv_c, "wo": wo_c, "tri": tri}
        if mask_mode == "full":
            im["maskt"] = maskt
        in_maps.append(im)
    return in_maps, NHL


def kernel(hidden_states, attention_mask, Wq, Wk, Wv, Wo):
    global LAST_RESULT
    hidden_states = np.asarray(hidden_states, dtype=np.float32)
    attention_mask = np.asarray(attention_mask, dtype=np.float32)
    Wq, Wk, Wv, Wo = (np.asarray(w, dtype=np.float32) for w in (Wq, Wk, Wv, Wo))
    B, S, H = hidden_states.shape

    mask_mode = _detect_mask_mode(attention_mask, S)
    in_maps, NHL = shard_inputs(hidden_states, attention_mask, Wq, Wk, Wv, Wo,
                                mask_mode)

    key = (B, S, H, NHL, mask_mode)
    if key not in _nc_cache:
        _nc_cache[key] = build_attn_core(B=B, S=S, H=H, NHL=NHL,
                                         mask_mode=mask_mode)
    nc = _nc_cache[key]

    res = run_bass_kernel_spmd(nc, in_maps, core_ids=list(range(N_CORES)),
                               trace=TRACE, trace_cores=TRACE_CORES)
    LAST_RESULT = res

    out = res.results[0]["out_p"].astype(np.float32)
    for c in range(1, N_CORES):
        out = out + res.results[c]["out_p"].astype(np.float32)
    return out.reshape(B, S, H)


# revision 18
# speedup vs baseline: 1.1499x; 1.0820x over previous
"""GQA causal attention (B=2, S=2048, H=2048, 32 Q heads / 8 KV heads, hd=64)
as an 8-way tensor-parallel Trainium2 Bass kernel.

Sharding: heads. Each NeuronCore gets 4 Q heads + their KV head (Wq/Wk/Wv
column slices, Wo row slice), computes a partial output over the full batch,
and the host sums the 8 bf16 partials (the Wo all-reduce done host-side).

v2 design (vs the fp32r baseline): everything bf16 on the PE, and the whole
kernel is ONE software-pipelined loop over 8 superblocks of 512 query
positions.  In slot i the instruction stream interleaves four stages --
ht prefetch for block i+2, projections of block i+1, attention of block i,
output projection of block i-1 -- so the tensor engine always has an
independent matmul ready and stays at its full (ramped) clock.  Causal
structure is exploited at matmul granularity: scores/exp/AV only touch
columns q >= k.

Per-core dataflow (d-major, no activation transposes except V):
    Q_T  = (Wq_c * scale)^T @ hidden^T          [256, B*S]   (heads stacked)
    K_T  = Wk_c^T @ hidden^T  (rows 0-63, duplicated to 64-127 for odd heads)
    V    = PE-transpose(Wv_c^T @ hidden^T)      [keys, 64] stored [V|1|V]
    S_T[k,q] = K_T(chunk)^T x Q_T               causal chunks only
    P_T  = exp(S_T + tri on diagonal chunks)    bf16
    ctx_aug = [V|1]^T @ P_T                     even heads -> psum rows 0-64
              [1|V]^T @ P_T                     odd heads  -> psum rows 63-127
    ctx  = ctx_aug * bcast(1/denom)             denom recip via [4,128] DVE
    out_partial = ctx^T @ Wo_c                  [B*S, 2048] bf16
"""

import sys

for _p in ("/root/.axon_site", "/root/.axon_site/_ro/trn_rl_repo",
           "/root/.axon_site/_ro/pypackages", "/opt/trn_rl_repo", "/opt/pypackages"):
    if _p not in sys.path:
        sys.path.append(_p)

from contextlib import ExitStack

import numpy as np

import concourse.bass as bass  # noqa: F401
import concourse.tile as tile
from concourse import bacc, mybir
from concourse.bass_utils import run_bass_kernel_spmd

F32 = mybir.dt.float32
BF16 = mybir.dt.bfloat16
P = 128
KC = 128
QT = 512
N_CORES = 8
HD = 64
NEG = -1e9

TRACE = False            # test harness flips this for NTFF profiling
TRACE_CORES = None
LAST_RESULT = None       # BassKernelResults of the last run (for the harness)

_nc_cache = {}


def build_attn_core(B=2, S=2048, H=2048, NHL=4, mask_mode="causal", debug_dump=False):
    """Build + bass-compile the per-core program.

    DRAM inputs (per core):
      ht  [H, B*S] bf16   hidden transposed      wq [H, NHL*HD] bf16 (pre-scaled)
      wkv [H, 2*HD] bf16  [Wk_c | Wv_c]          wo [NHL*HD, H] bf16
      tri [KC, KC] f32    transposed causal block mask (tri[k,q]=0 iff k<=q)
      maskt [B, S, S] f32 (only mask_mode=="full") additive mask transposed
    Output: out_p [B*S, H] bf16.
    """
    NQ = B * S
    CL = NHL * HD                       # 256 q-head cols per core
    assert H % P == 0 and S % QT == 0
    NHC = H // P                        # 16 contraction chunks
    NCC = CL // P                       # 2 head-pair groups
    QPB = S // QT                       # 4 q-blocks per batch
    NBLK = B * QPB                      # 8 superblocks
    KPB = S // KC                       # 16 key chunks per batch
    DPT = QT // KC                      # 4 key chunks per q-block
    EXP = mybir.ActivationFunctionType.Exp
    CPY = mybir.ActivationFunctionType.Copy

    nc = bacc.Bacc("TRN2", target_bir_lowering=False, debug=False)

    ht = nc.dram_tensor("ht", [H, NQ], BF16, kind="ExternalInput").ap()
    wq = nc.dram_tensor("wq", [H, CL], BF16, kind="ExternalInput").ap()
    wkv = nc.dram_tensor("wkv", [H, 2 * HD], BF16, kind="ExternalInput").ap()
    wo = nc.dram_tensor("wo", [CL, H], BF16, kind="ExternalInput").ap()
    tri = nc.dram_tensor("tri", [KC, KC], F32, kind="ExternalInput").ap()
    if mask_mode == "full":
        maskt = nc.dram_tensor("maskt", [B, S, S], F32, kind="ExternalInput").ap()
    out_p = nc.dram_tensor("out_p", [NQ, H], BF16, kind="ExternalOutput").ap()
    dscr = nc.dram_tensor("dscr", [NBLK * NHL, QT], F32, kind="Internal").ap()
    if debug_dump:
        dbg_qt = nc.dram_tensor("dbg_qt", [NCC, P, NQ], BF16, kind="ExternalOutput").ap()
        dbg_kt = nc.dram_tensor("dbg_kt", [P, NQ], BF16, kind="ExternalOutput").ap()
        dbg_v = nc.dram_tensor("dbg_v", [P, NQ // KC, HD + 1], BF16, kind="ExternalOutput").ap()
        dbg_ctx = nc.dram_tensor("dbg_ctx", [NBLK, P, NCC, QT], BF16, kind="ExternalOutput").ap()
        dbg_rb = nc.dram_tensor("dbg_rb", [NBLK * NHL, QT], F32, kind="ExternalOutput").ap()
    dscr2 = nc.dram_tensor("dscr2", [NBLK * NHL, QT], F32, kind="Internal").ap()

    ht_r = ht.rearrange("(o p) m -> p o m", p=P)      # [128, 16, 4096]

    with tile.TileContext(nc) as tc, ExitStack() as ctx:
        # ---- persistent SBUF ----
        pers = ctx.enter_context(tc.tile_pool(name="pers", bufs=1))
        wq_sb = pers.tile([P, NHC, CL], BF16, tag="wq")
        wkv_sb = pers.tile([P, NHC, 2 * HD], BF16, tag="wkv")
        wo_sb = pers.tile([P, NCC, H], BF16, tag="wo")
        tri_sb = pers.tile([KC, KC], F32, tag="tri")
        # weight loads split so the first projection can start early
        wq_r = wq.rearrange("(o p) m -> p o m", p=P)
        wkv_r = wkv.rearrange("(o p) m -> p o m", p=P)
        wo_r = wo.rearrange("(o p) m -> p o m", p=P)

        # identity (bf16) for PE transposes of V
        ident = pers.tile([P, P], BF16, tag="ident")
        nc.gpsimd.memset(ident[:], 1.0)
        nc.gpsimd.affine_select(
            out=ident[:], in_=ident[:],
            compare_op=mybir.AluOpType.is_equal, fill=0.0,
            base=0, pattern=[[-1, P]], channel_multiplier=1,
        )

        qt_sb = [pers.tile([P, NQ], BF16, tag=f"qt{c}", name=f"qt{c}")
                 for c in range(NCC)]
        kt_sb = pers.tile([P, NQ], BF16, tag="kt")          # [K_T ; K_T]
        v_sb = pers.tile([P, NQ // KC, HD + 1], BF16, tag="v")      # [V|1]
        nc.gpsimd.memset(v_sb[:, :, HD], 1.0)
        ctx_sb = pers.tile([P, 2, NCC, QT], BF16, tag="ctx")

        # ---- SBUF pools ----
        hpool = ctx.enter_context(tc.tile_pool(name="hpool", bufs=2))
        vt_pool = ctx.enter_context(tc.tile_pool(name="vtp", bufs=2))
        pt_pool = ctx.enter_context(tc.tile_pool(name="ptp", bufs=6))
        dpool = ctx.enter_context(tc.tile_pool(name="dpool", bufs=3))
        bcpool = ctx.enter_context(tc.tile_pool(name="bcp", bufs=2))
        obpool = ctx.enter_context(tc.tile_pool(name="obp", bufs=3))
        if mask_mode == "full":
            mpool = ctx.enter_context(tc.tile_pool(name="mpool", bufs=4))

        # ---- PSUM pool (tags: pq 1, pkv 1, sps 2, cps 2, pow 2) ----
        psum = ctx.enter_context(tc.tile_pool(name="psum", bufs=1, space="PSUM"))

        ht_tiles = {}                   # blk -> [4 x tile [128,4,512]]

        # ================= stream generators =================
        # Each stream yields (kind, closure); emission interleaves streams.

        def prefetch_stream(blk):
            """Issue the 4 coarse ht DMAs for superblock blk."""
            g0 = blk * QT
            tiles = []
            for g in range(4):
                h4 = hpool.tile([P, 4, QT], BF16, tag=f"h{g}", name=f"h4_{g}")
                tiles.append(h4)
            ht_tiles[blk] = tiles

            def mk(g):
                def emit():
                    nc.sync.dma_start(tiles[g][:],
                                      ht_r[:, g * 4:(g + 1) * 4, g0:g0 + QT])
                return emit
            for g in range(4):
                yield ('dma', mk(g))

        def proj_stream(blk):
            """Projections of superblock blk -> qt_sb/kt_sb/v_sb columns.

            Q runs as two sequential accumulation groups (heads 0,1 then
            heads 2,3) sharing one PSUM bank, freeing a bank for a third
            scores slot."""
            g0 = blk * QT
            hts = ht_tiles[blk]
            pqA = psum.tile([P, QT], F32, tag="pq", bufs=1, name="pqA")
            pkv = psum.tile([P, QT], F32, tag="pkv", bufs=1, name="pkv")

            def mk_mmA(hc, pqA=pqA, pkv=pkv):
                def emit():
                    mv = hts[hc // 4][:, hc % 4, :]
                    fl = dict(start=(hc == 0), stop=(hc == NHC - 1))
                    nc.tensor.matmul(pqA[:], wq_sb[:, hc, 0:P], mv, **fl)
                    nc.tensor.matmul(pkv[:], wkv_sb[:, hc, :], mv, **fl)
                return emit
            for hc in range(NHC):
                yield ('mmA', mk_mmA(hc))

            vtmp = vt_pool.tile([P, QT], BF16, tag="vt", name="vtmp")

            def drainA(pqA=pqA, pkv=pkv, vtmp=vtmp):
                nc.vector.tensor_copy(qt_sb[0][:, g0:g0 + QT], pqA[:])
                nc.vector.tensor_copy(kt_sb[:HD, g0:g0 + QT], pkv[:HD, :])
                nc.vector.tensor_copy(vtmp[HD:2 * HD, :], pkv[HD:2 * HD, :])
            yield ('drainA', drainA)

            def mk_tr(j, vtmp=vtmp):
                kcg = g0 // KC + j

                def emit():
                    tp = psum.tile([P, HD], BF16, tag="pkv", bufs=1, name="tp")
                    nc.tensor.transpose(
                        tp[:, :HD],
                        vtmp[HD:2 * HD, j * KC:(j + 1) * KC],
                        ident[HD:2 * HD, HD:2 * HD],
                    )
                    nc.vector.tensor_copy(v_sb[:, kcg, :HD], tp[:, :HD])
                return emit
            for j in range(DPT):
                yield ('tr', mk_tr(j))
            # duplicate K rows 0-63 -> 64-127 for odd heads
            yield ('dma', lambda: nc.gpsimd.dma_start(
                kt_sb[HD:2 * HD, g0:g0 + QT], kt_sb[:HD, g0:g0 + QT]))

            pqB = psum.tile([P, QT], F32, tag="pq", bufs=1, name="pqB")

            def mk_mmB(hc, pqB=pqB):
                def emit():
                    mv = hts[hc // 4][:, hc % 4, :]
                    fl = dict(start=(hc == 0), stop=(hc == NHC - 1))
                    nc.tensor.matmul(pqB[:], wq_sb[:, hc, P:CL], mv, **fl)
                return emit
            for hc in range(NHC):
                yield ('mmB', mk_mmB(hc))

            def drainB(pqB=pqB):
                nc.vector.tensor_copy(qt_sb[1][:, g0:g0 + QT], pqB[:])
            yield ('drainB', drainB)

        def attn_stream(blk):
            """Attention of superblock blk into ctx_sb[:, blk%2]."""
            b, qtb = blk // QPB, blk % QPB
            g0 = blk * QT
            ib = blk % 2
            nkc = (qtb + 1) * DPT if mask_mode == "causal" else KPB
            for h in range(NHL):
                hb = (h % 2) * HD
                cc = h // 2
                even = (h % 2 == 0)
                cps = psum.tile([P, QT], F32, tag="cps", bufs=2, name="cps")
                pend = []   # deferred AV emissions (one-unit lag)

                def emit_av(item, cps=cps, nkc=nkc):
                    akc, alo, apt, akcg = item
                    nc.tensor.matmul(cps[0:HD + 1, alo:], v_sb[:, akcg, :],
                                     apt[:, alo:],
                                     start=(akc == 0), stop=(akc == nkc - 1))

                def mk_unit(kc, hb=hb, cc=cc, pend=pend, emit_av=emit_av, b=b):
                    kcg = b * KPB + kc
                    do = kc * KC - qtb * QT if mask_mode == "causal" else -1
                    lo = max(do, 0)

                    def emit():
                        sps = psum.tile([P, QT], F32, tag="sps", bufs=3,
                                        name="sps")
                        nc.tensor.matmul(
                            sps[:, lo:],
                            kt_sb[hb:hb + HD, kcg * KC:(kcg + 1) * KC],
                            qt_sb[cc][hb:hb + HD, g0 + lo:g0 + QT],
                            start=True, stop=True,
                        )
                        if len(pend) >= 2:
                            emit_av(pend.pop(0))
                        if mask_mode == "full":
                            mt = mpool.tile([KC, QT], F32, tag="mt", name="mt")
                            nc.sync.dma_start(
                                mt[:], maskt[b, kc * KC:(kc + 1) * KC,
                                             (g0 - b * S):(g0 - b * S) + QT])
                            nc.vector.tensor_add(sps[:], sps[:], mt[:])
                        elif do >= 0:
                            nc.vector.tensor_add(
                                sps[:, do:do + KC], sps[:, do:do + KC], tri_sb[:])
                        pt = pt_pool.tile([P, QT], BF16, tag="pt", name="pt")
                        nc.scalar.activation(pt[:, lo:], sps[:, lo:], EXP)
                        pend.append((kc, lo, pt, kcg))
                    return emit

                for kc in range(nkc):
                    yield ('attn', mk_unit(kc))

                def finalize(h=h, cc=cc, even=even, cps=cps, pend=pend,
                             emit_av=emit_av, ib=ib, blk=blk):
                    while pend:
                        emit_av(pend.pop(0))
                    # normalize: denom row -> dram -> [4,128] -> recip -> dram
                    # -> [1,512] -> broadcast.  (SBUF partition-reshape DMAs
                    # are illegal; the DRAM bounce is the legal spelling.)
                    hh = blk * NHL + h
                    den = dpool.tile([P, QT], F32, tag="den", name="den")
                    nc.scalar.activation(den[HD:HD + 1, :],
                                         cps[HD:HD + 1, :], CPY)
                    nc.gpsimd.dma_start(dscr[hh:hh + 1, :], den[HD:HD + 1, :])
                    dh = dpool.tile([4, KC], F32, tag="dh", name="dh")
                    nc.gpsimd.dma_start(
                        dh[:],
                        dscr[hh:hh + 1, :].rearrange("o (a b) -> (o a) b", a=4))
                    rc = dpool.tile([4, KC], F32, tag="rc", name="rc")
                    nc.vector.reciprocal(rc[:], dh[:])
                    nc.gpsimd.dma_start(
                        dscr2[hh:hh + 1, :].rearrange("o (a b) -> (o a) b", a=4),
                        rc[:])
                    rb = dpool.tile([1, QT], F32, tag="rb", name="rb")
                    nc.gpsimd.dma_start(rb[:], dscr2[hh:hh + 1, :])
                    if debug_dump:
                        nc.sync.dma_start(dbg_rb[hh:hh + 1, :], rb[:])
                    bc = bcpool.tile([P, QT], F32, tag="bc", name="bc")
                    nc.gpsimd.partition_broadcast(bc[0:HD, :], rb[:])
                    if even:
                        nc.vector.tensor_mul(ctx_sb[0:HD, ib, cc, :],
                                             cps[0:HD, :], bc[0:HD, :])
                    else:
                        ctmp = bcpool.tile([HD, QT], BF16, tag="ctmp",
                                           name="ctmp")
                        nc.vector.tensor_mul(ctmp[:], cps[0:HD, :], bc[0:HD, :])
                        nc.gpsimd.dma_start(ctx_sb[HD:2 * HD, ib, cc, :],
                                            ctmp[:])
                yield ('fin', finalize)

        def wo_stream(blk):
            """Output projection of superblock blk from ctx_sb[:, blk%2]."""
            ib = blk % 2
            r0 = blk * QT
            ET = 512
            last_blk = (blk == NBLK - 1)
            if debug_dump:
                yield ('dbg', lambda: nc.sync.dma_start(
                    dbg_ctx[blk], ctx_sb[:, ib, :, :]))
            for qc in range(QT // P):
                ob = obpool.tile([P, H], BF16, tag="ob", name="ob")

                def mk_unit(et, qc=qc, ob=ob):
                    def emit():
                        if last_blk:
                            # attention is done: cps/sps banks are free, so
                            # rotate through 3 banks to avoid drain stalls
                            tg = ["pow", "cps", "sps"][(qc * 4 + et) % 3]
                            bufs = {"pow": 1, "cps": 2, "sps": 3}[tg]
                        else:
                            tg, bufs = "pow", 1
                        po = psum.tile([P, ET], F32, tag=tg, bufs=bufs,
                                       name="po")
                        for cc2 in range(NCC):
                            nc.tensor.matmul(
                                po[:],
                                ctx_sb[:, ib, cc2, qc * P:(qc + 1) * P],
                                wo_sb[:, cc2, et * ET:(et + 1) * ET],
                                start=(cc2 == 0), stop=(cc2 == NCC - 1),
                            )
                        dst = ob[:, et * ET:(et + 1) * ET]
                        nc.vector.tensor_copy(dst, po[:])
                        if last_blk:
                            nc.gpsimd.dma_start(
                                out_p[r0 + qc * P:r0 + (qc + 1) * P,
                                      et * ET:(et + 1) * ET], dst)
                        elif et % 2 == 1:
                            nc.gpsimd.dma_start(
                                out_p[r0 + qc * P:r0 + (qc + 1) * P,
                                      (et - 1) * ET:(et + 1) * ET],
                                ob[:, (et - 1) * ET:(et + 1) * ET])
                    return emit
                for et in range(H // ET):
                    yield ('wo', mk_unit(et))

        # ================= merge + emit =================
        def merge(streams):
            """Proportional interleave of unit streams (virtual-time merge)."""
            lists = [list(s) for s in streams if s is not None]
            lists = [l for l in lists if l]
            idx = [0] * len(lists)
            while True:
                best, bestv = -1, 2.0
                for j, l in enumerate(lists):
                    if idx[j] < len(l):
                        v = (idx[j] + 0.5) / len(l)
                        if v < bestv:
                            best, bestv = j, v
                if best < 0:
                    break
                lists[best][idx[best]][1]()
                idx[best] += 1

        # prologue: fine-grained ht loads for blocks 0-1 issued FIRST so
        # the first matmuls start early; weights follow (wq interleaved
        # early, wo deferred -- it is not needed until slot 1).
        for blk0 in range(2):
            tiles0 = [hpool.tile([P, 4, QT], BF16, tag=f"h{g}", name=f"h4p_{g}")
                      for g in range(4)]
            ht_tiles[blk0] = tiles0
            for g in range(4):
                for j in range(4):
                    nc.sync.dma_start(tiles0[g][:, j, :],
                                      ht_r[:, g * 4 + j, blk0 * QT:(blk0 + 1) * QT])
            if blk0 == 0:
                for g in range(8):
                    nc.sync.dma_start(wq_sb[:, g * 2:(g + 1) * 2, :],
                                      wq_r[:, g * 2:(g + 1) * 2, :])
                for g in range(2):
                    nc.sync.dma_start(wkv_sb[:, g * 8:(g + 1) * 8, :],
                                      wkv_r[:, g * 8:(g + 1) * 8, :])
                nc.sync.dma_start(tri_sb[:], tri)
        for g in range(2):
            for g2 in range(2):
                nc.sync.dma_start(wo_sb[:, g, g2 * (H // 2):(g2 + 1) * (H // 2)],
                                  wo_r[:, g, g2 * (H // 2):(g2 + 1) * (H // 2)])
        for _, emit in proj_stream(0):
            emit()
        # main loop: attention trails projections by one slot
        for i in range(NBLK):
            streams = [attn_stream(i)]
            if i + 1 < NBLK:
                streams.append(proj_stream(i + 1))
            if i + 2 < NBLK:
                streams.append(prefetch_stream(i + 2))
            if i - 1 >= 0:
                streams.append(wo_stream(i - 1))
            merge(streams)
        # epilogue
        for _, emit in wo_stream(NBLK - 1):
            emit()
        if debug_dump:
            for c in range(NCC):
                nc.sync.dma_start(dbg_qt[c], qt_sb[c][:])
            nc.sync.dma_start(dbg_kt[:], kt_sb[:])
            nc.sync.dma_start(dbg_v[:], v_sb[:])


    nc.compile()
    return nc


def _detect_mask_mode(m, S):
    if not np.any(m):
        return "zeros"
    b0 = np.asarray(m[0, 0])
    qi = np.arange(S)
    tl = qi[None, :] <= qi[:, None]
    if (b0[tl] == 0.0).all() and (b0[~tl] <= -1e8).all() and (m == b0).all():
        return "causal"
    return "full"


def shard_inputs(hidden_states, attention_mask, Wq, Wk, Wv, Wo, mask_mode):
    import ml_dtypes
    bf16 = ml_dtypes.bfloat16
    B, S, H = hidden_states.shape
    NH = Wq.shape[1] // HD
    NKV = Wk.shape[1] // HD
    NHL = NH // N_CORES
    scale = np.float32(1.0 / np.sqrt(HD))

    ht = np.ascontiguousarray(
        hidden_states.reshape(B * S, H).T).astype(bf16)
    ki = np.arange(KC)
    tri = np.where(ki[:, None] <= ki[None, :], 0.0, NEG).astype(np.float32)
    if mask_mode == "full":
        maskt = np.ascontiguousarray(
            np.asarray(attention_mask)[:, 0].transpose(0, 2, 1).astype(np.float32))

    in_maps = []
    for c in range(N_CORES):
        wq_c = np.ascontiguousarray(
            Wq[:, c * NHL * HD:(c + 1) * NHL * HD] * scale).astype(bf16)
        kv0 = c * (NKV // N_CORES) * HD
        wkv_c = np.ascontiguousarray(np.concatenate(
            [Wk[:, kv0:kv0 + HD], Wv[:, kv0:kv0 + HD]], axis=1)).astype(bf16)
        wo_c = np.ascontiguousarray(
            Wo[c * NHL * HD:(c + 1) * NHL * HD, :]).astype(bf16)
        im = {"ht": ht, "wq": wq_c, "wkv": wkv_c, "wo": wo_c, "tri": tri}
        if mask_mode == "full":
            im["maskt"] = maskt
        in_maps.append(im)
    return in_maps, NHL


def kernel(hidden_states, attention_mask, Wq, Wk, Wv, Wo):
    global LAST_RESULT
    hidden_states = np.asarray(hidden_states, dtype=np.float32)
    attention_mask = np.asarray(attention_mask, dtype=np.float32)
    Wq, Wk, Wv, Wo = (np.asarray(w, dtype=np.float32) for w in (Wq, Wk, Wv, Wo))
    B, S, H = hidden_states.shape

    mask_mode = _detect_mask_mode(attention_mask, S)
    in_maps, NHL = shard_inputs(hidden_states, attention_mask, Wq, Wk, Wv, Wo,
                                mask_mode)

    key = (B, S, H, NHL, mask_mode)
    if key not in _nc_cache:
        _nc_cache[key] = build_attn_core(B=B, S=S, H=H, NHL=NHL,
                                         mask_mode=mask_mode)
    nc = _nc_cache[key]

    res = run_bass_kernel_spmd(nc, in_maps, core_ids=list(range(N_CORES)),
                               trace=TRACE, trace_cores=TRACE_CORES)
    LAST_RESULT = res

    out = res.results[0]["out_p"].astype(np.float32)
    for c in range(1, N_CORES):
        out = out + res.results[c]["out_p"].astype(np.float32)
    return out.reshape(B, S, H)
